# revision 20
# baseline (speedup 1.0000x reference)
"""Causal single-head self-attention on 8 Trainium2 NeuronCores.

Problem: x:[8,2048,1024], Wq/Wk/Wv:[1024,64] ->
    out[b] = softmax(tril(x[b]Wq (x[b]Wk)^T / 64)) @ (x[b]Wv)   [8,2048,64]

Sharding: data-parallel over batch -- core b gets batch element b.
Weights replicated.

Per-core algorithm (fp32 matmuls on TRN2 lower to 2x LOW_HIGH PE passes,
so all matmul operands are bf16 with fp32 PSUM accumulation; measured
end-to-end rel err ~3.8e-3 against the fp32 reference):
  - host pre-swizzles x[b] into the exact SBUF layout xp[128, 8, 2048]
    (partition, e-chunk, seq) in bf16 so every DMA is dense, and
    likewise the weights; kernel output is out^T [64, S] fp32,
    un-transposed on the host
  - per q-block of 512 (pipelined with the xT DMA):
      qkT[128, qb]: rows 0:64 = q^T, 64:128 = k^T via packed projection
      (lhsT=[Wq|Wk][e]); kT DMA-shifted to partitions 0:64 and qT
      DMA-shifted up to partitions 64:128 so score matmuls can be
      row-packed two-at-a-time on PE row groups 0:64 / 64:128;
      v^T projected likewise then PE-transposed to natural v[s,64] bf16
      with a ones column appended -> v_aug[s, 65]
      attention: scores^T[kc, qb] = kT_kc.T @ qT_qb (K=64, fp32 psum);
      exp via ACT (scale=1/64, fp32 in, bf16 out); causal = skip
      above-diagonal chunks + truncate diagonal chunks' q-range +
      gpsimd affine_select triangular mask; out^T psum[65, qb] +=
      v_aug[kc].T @ expT; row 64 accumulates the softmax denominators
      normalize: reciprocal of row 64 -> broadcast over partitions 0:64
      via a ones[64,64] matmul -> elementwise multiply -> out^T store
"""

import os
from contextlib import ExitStack

import numpy as np

import concourse.bass as bass
import concourse.mybir as mybir
import concourse.tile as tile
from concourse import bacc
from concourse.bass_utils import run_bass_kernel_spmd
from concourse.masks import make_identity

B, S, E, H = 8, 2048, 1024, 64
P = 128
QB = 512  # q-block (psum free dim)
F32 = mybir.dt.float32
BF16 = mybir.dt.bfloat16


def build_kernel_body(tc, xp_d, wqk_d, wv_d, out_d, s=S, e_dim=E):
    nc = tc.nc
    EC = e_dim // P  # e-chunks
    NQB = s // QB    # q-blocks
    NT = s // P      # s-tiles of 128
    KPQ = QB // P    # k-chunks per q-block (4)

    ctx = ExitStack()
    with ctx:
        const = ctx.enter_context(tc.tile_pool(name="const", bufs=1))
        big = ctx.enter_context(tc.tile_pool(name="big", bufs=1))

        # weights on the ACT HWDGE ring so the Sync ring starts on x
        wqk_sb = const.tile([P, EC, 2 * H], BF16)
        nc.scalar.dma_start(wqk_sb[:], wqk_d[:])
        wv_sb = const.tile([P, EC, H], BF16)
        nc.scalar.dma_start(wv_sb[:], wv_d[:])
        ident_bf = const.tile([P, P], BF16)
        make_identity(nc, ident_bf[:])
        ones_sb = const.tile([H, H], F32)
        nc.gpsimd.memset(ones_sb[:], 1.0)
        recb_sb = const.tile([H, QB], F32)
        nc.gpsimd.memset(recb_sb[:], 0.0)  # rows 1:64 stay zero

        # x blocks q-block-major; first q-block split so the first
        # projection matmul can start after 256 KB
        xp_sb = big.tile([P, EC, s], BF16)
        for qb in range(NQB):
            for g in range(EC // 2):
                nc.sync.dma_start(
                    xp_sb[:, 2 * g:2 * g + 2, qb * QB:(qb + 1) * QB],
                    xp_d[:, 2 * g:2 * g + 2, qb * QB:(qb + 1) * QB])

        qkT_sb = big.tile([P, s], BF16)  # rows 0:64 qT, rows 64:128 kT
        kT_sb = big.tile([H, s], BF16)   # kT at base partition 0
        qT2_sb = big.tile([P, s], BF16)  # qT duplicated at rows 64:128
        vT_sb = big.tile([H, s], BF16)
        v_sb = big.tile([P, NT, H + 1], BF16)  # natural v + ones col
        outT_sb = big.tile([H, s], F32)

        nc.gpsimd.memset(v_sb[:, :, H:H + 1], 1.0)

        # PSUM budget (8 banks): pqk 1 + pvt 1 + ps 3 (also serves the
        # bcast matmul) + po 2 + ptr 1 = 8
        pqk = ctx.enter_context(tc.tile_pool(name="ps_qk", bufs=1, space="PSUM"))
        pvt = ctx.enter_context(tc.tile_pool(name="ps_vt", bufs=1, space="PSUM"))
        ps = ctx.enter_context(tc.tile_pool(name="ps_s", bufs=3, space="PSUM"))
        po = ctx.enter_context(tc.tile_pool(name="ps_o", bufs=2, space="PSUM"))
        ptr = ctx.enter_context(tc.tile_pool(name="ps_tr", bufs=1, space="PSUM"))
        ep = ctx.enter_context(tc.tile_pool(name="expp", bufs=6))
        sp = ctx.enter_context(tc.tile_pool(name="smalls", bufs=4))

        psum_o_pend = [None] * NQB

        def normalize(qb):
            # psum_o[65, QB]: rows 0:64 = unnormalized out^T, row 64 =
            # softmax denominators. 1/den = exp(-ln(den)) on ACT (the
            # DVE reciprocal streams the free dim at ~6.5 cyc/elem on a
            # single lane -- 3.4 us for 512), then broadcast over
            # partitions via ones[64,64] @ [recip; zeros], multiply,
            # store out^T.
            qsl = slice(qb * QB, (qb + 1) * QB)
            psum_o = psum_o_pend[qb]
            nc.vector.reciprocal(recb_sb[0:1, :], psum_o[H:H + 1, :])
            psum_b = ps.tile([P, QB], F32, tag="sc")
            nc.tensor.matmul(
                psum_b[0:H, :], lhsT=ones_sb[:], rhs=recb_sb[:],
                start=True, stop=True)
            bcast = sp.tile([H, QB], F32, tag="bc")
            nc.vector.tensor_copy(bcast[:], psum_b[0:H, :])
            nc.vector.tensor_mul(
                out=outT_sb[:, qsl], in0=psum_o[0:H, :], in1=bcast[:])
            nc.sync.dma_start(out_d[:, qsl], outT_sb[:, qsl])

        for qb in range(NQB):
            qsl = slice(qb * QB, (qb + 1) * QB)
            # ---- projections for this q-block ----
            psum_qk = pqk.tile([P, QB], F32, tag="qk")
            psum_vT = pvt.tile([H, QB], F32, tag="vt")
            for ec in range(EC):
                nc.tensor.matmul(
                    psum_qk[:], lhsT=wqk_sb[:, ec, :],
                    rhs=xp_sb[:, ec, qsl],
                    start=(ec == 0), stop=(ec == EC - 1))
                nc.tensor.matmul(
                    psum_vT[:], lhsT=wv_sb[:, ec, :],
                    rhs=xp_sb[:, ec, qsl],
                    start=(ec == 0), stop=(ec == EC - 1))
            nc.vector.tensor_copy(qkT_sb[:, qsl], psum_qk[:])
            nc.gpsimd.dma_start(kT_sb[:, qsl], qkT_sb[H:P, qsl])
            nc.gpsimd.dma_start(qT2_sb[H:P, qsl], qkT_sb[0:H, qsl])
            nc.vector.tensor_copy(vT_sb[:, qsl], psum_vT[:])
            for t in range(qb * KPQ, (qb + 1) * KPQ):
                pvtr = ptr.tile([P, H], BF16, tag="tr")
                nc.tensor.transpose(
                    pvtr[:], vT_sb[:, t * P:(t + 1) * P], ident_bf[0:H, 0:H])
                nc.vector.tensor_copy(v_sb[:, t, 0:H], pvtr[:])

            # previous q-block's normalize overlaps this one's attention
            if qb > 0:
                normalize(qb - 1)

            # ---- attention for this q-block ----
            nkc = (qb + 1) * KPQ
            psum_o = po.tile([H + 1, QB], F32)
            psum_o_pend[qb] = psum_o
            for pr in range(nkc // 2):
                kc0, kc1 = 2 * pr, 2 * pr + 1
                # row-packed pair: kc0 on PE rows 0:64, kc1 on rows
                # 64:128 (kT lives at rows 64:128 of qkT_sb; qT
                # duplicated there); the two matmuls run concurrently
                psum_prs = []
                offs = []
                for i, kc in enumerate((kc0, kc1)):
                    o = max(0, kc * P - qb * QB)
                    offs.append(o)
                    psum_s = ps.tile([P, QB], F32, tag="sc")
                    psum_prs.append(psum_s)
                    if i == 0:
                        nc.tensor.matmul(
                            psum_s[:, o:],
                            lhsT=kT_sb[:, kc * P:(kc + 1) * P],
                            rhs=qkT_sb[0:H, qsl][:, o:],
                            start=True, stop=True)
                    else:
                        nc.tensor.matmul(
                            psum_s[:, o:],
                            lhsT=qkT_sb[H:P, kc * P:(kc + 1) * P],
                            rhs=qT2_sb[H:P, qsl][:, o:],
                            start=True, stop=True)
                for i, (kc, o) in enumerate(((kc0, offs[0]), (kc1, offs[1]))):
                    et = ep.tile([P, QB], BF16)
                    nc.scalar.activation(
                        et[:, o:], psum_prs[i][:, o:],
                        mybir.ActivationFunctionType.Exp, scale=1.0 / H)
                    if kc * P - qb * QB >= 0:
                        # diagonal chunk: keep where q >= k (j - p >= 0)
                        nc.gpsimd.affine_select(
                            out=et[:, o:], in_=et[:, o:],
                            compare_op=mybir.AluOpType.is_ge,
                            fill=0.0, base=0,
                            channel_multiplier=-1,
                            pattern=[[1, QB - o]])
                    nc.tensor.matmul(
                        psum_o[:, o:],
                        lhsT=v_sb[:, kc, :],
                        rhs=et[:, o:],
                        start=(kc == 0), stop=(kc == nkc - 1))

        normalize(NQB - 1)


def build_bass(s=S, e_dim=E, n_cores=B):
    nc = bacc.Bacc(
        "TRN2", target_bir_lowering=False, debug=False, num_devices=n_cores)
    EC = e_dim // P
    xp_d = nc.dram_tensor("xp", [P, EC, s], BF16, kind="ExternalInput").ap()
    wqk_d = nc.dram_tensor(
        "wqk", [P, EC, 2 * H], BF16, kind="ExternalInput").ap()
    wv_d = nc.dram_tensor("wv", [P, EC, H], BF16, kind="ExternalInput").ap()
    out_d = nc.dram_tensor("out", [H, s], F32, kind="ExternalOutput").ap()
    with tile.TileContext(nc) as tc:
        build_kernel_body(tc, xp_d, wqk_d, wv_d, out_d, s=s, e_dim=e_dim)
    nc.compile()
    return nc


_nc_cache = None


def _ensure_ntff_hook():
    """Dev-only: provide the antenv.axon_hooks shim so trace=True can
    capture NTFF profiles through libaxon_pjrt.so in this container."""
    import sys
    import types
    import ctypes
    import contextlib

    try:
        from antenv.axon_hooks import get_axon_ntff_profile_hook  # noqa
        return
    except ImportError:
        pass
    import antenv

    mod = types.ModuleType("antenv.axon_hooks")
    _h = [None]
    mod.set_axon_ntff_profile_hook = lambda h: _h.__setitem__(0, h)
    mod.get_axon_ntff_profile_hook = lambda: _h[0]
    sys.modules["antenv.axon_hooks"] = mod
    antenv.axon_hooks = mod

    so_path = "/opt/axon/libaxon_pjrt.so"
    lib = ctypes.CDLL(so_path)
    if not hasattr(lib, "axon_start_nrt_profile"):
        return
    lib.axon_start_nrt_profile.argtypes = [
        ctypes.POINTER(ctypes.c_int64), ctypes.c_size_t]
    lib.axon_start_nrt_profile.restype = ctypes.c_int64
    lib.axon_stop_nrt_profile.argtypes = [ctypes.c_char_p]
    lib.axon_stop_nrt_profile.restype = ctypes.c_int64

    @contextlib.contextmanager
    def _hook(output_dir, device_ids):
        import jax
        jax.devices()
        if device_ids:
            ids = (ctypes.c_int64 * len(device_ids))(*device_ids)
            rc = lib.axon_start_nrt_profile(ids, len(device_ids))
        else:
            rc = lib.axon_start_nrt_profile(None, 0)
        if rc != 0:
            raise RuntimeError(f"axon_start_nrt_profile rc={rc}")
        try:
            yield
        finally:
            n = lib.axon_stop_nrt_profile(str(output_dir).encode())
            print(f"profile: {n} file(s) written to {output_dir}")

    mod.set_axon_ntff_profile_hook(_hook)

    # no bucket access in this container; keep artifacts local
    import concourse.bass_utils as bu
    bu.upload_artifacts = lambda tmpdir: tmpdir


def _swizzle(a, ec, p):
    """[E, M] -> [P, EC, M] with [pp, c, m] = a[c*p + pp, m]."""
    return np.ascontiguousarray(a.reshape(ec, p, a.shape[-1]).transpose(1, 0, 2))


def kernel(x, Wq, Wk, Wv):
    global _nc_cache
    import ml_dtypes
    bf = ml_dtypes.bfloat16

    x = np.asarray(x, dtype=np.float32)
    Wq = np.asarray(Wq, dtype=np.float32)
    Wk = np.asarray(Wk, dtype=np.float32)
    Wv = np.asarray(Wv, dtype=np.float32)

    if _nc_cache is None:
        _nc_cache = build_bass()
    nc = _nc_cache

    EC = E // P
    wqk = _swizzle(np.concatenate([Wq, Wk], axis=1).astype(bf), EC, P)
    wv = _swizzle(Wv.astype(bf), EC, P)
    in_maps = []
    for b in range(B):
        in_maps.append({
            "xp": _swizzle(x[b].T.astype(bf), EC, P),
            "wqk": wqk,
            "wv": wv,
        })

    trace = bool(int(os.environ.get("ATTN_TRACE", "0")))
    if trace:
        _ensure_ntff_hook()
    res = run_bass_kernel_spmd(
        nc, in_maps, core_ids=list(range(B)), trace=trace)
    if trace and res.exec_time_ns is not None:
        print(f"HW exec time: {res.exec_time_ns} ns")
        kernel.last_exec_time_ns = res.exec_time_ns
        kernel.last_results = res
    # out^T [64, S] per core -> [B, S, 64]
    out = np.stack(
        [np.ascontiguousarray(res.results[b]["out"].T) for b in range(B)],
        axis=0)
    return out


# revision 25
# speedup vs baseline: 1.0825x; 1.0825x over previous
"""Causal single-head self-attention on 8 Trainium2 NeuronCores.

Problem: x:[8,2048,1024], Wq/Wk/Wv:[1024,64] ->
    out[b] = softmax(tril(x[b]Wq (x[b]Wk)^T / 64)) @ (x[b]Wv)   [8,2048,64]

Sharding: data-parallel over batch -- core b gets batch element b.
Weights replicated.

Per-core algorithm (fp32 matmuls on TRN2 lower to 2x LOW_HIGH PE passes,
so all matmul operands are bf16 with fp32 PSUM accumulation; measured
end-to-end rel err ~3.8e-3 against the fp32 reference):
  - host pre-swizzles x[b] into the exact SBUF layout xp[128, 8, 2048]
    (partition, e-chunk, seq) in bf16 so every DMA is dense, and
    likewise the weights; kernel output is out^T [64, S] fp32,
    un-transposed on the host
  - per q-block of 512 (pipelined with the xT DMA):
      qkT[128, qb]: rows 0:64 = q^T, 64:128 = k^T via packed projection
      (lhsT=[Wq|Wk][e]); kT DMA-shifted to partitions 0:64 and qT
      DMA-shifted up to partitions 64:128 so score matmuls can be
      row-packed two-at-a-time on PE row groups 0:64 / 64:128;
      v^T projected likewise then PE-transposed to natural v[s,64] bf16
      with a ones column appended -> v_aug[s, 65]
      attention: scores^T[kc, qb] = kT_kc.T @ qT_qb (K=64, fp32 psum);
      exp via ACT (scale=1/64, fp32 in, bf16 out); causal = skip
      above-diagonal chunks + truncate diagonal chunks' q-range +
      gpsimd affine_select triangular mask; out^T psum[65, qb] +=
      v_aug[kc].T @ expT; row 64 accumulates the softmax denominators
      normalize: reciprocal of row 64 -> broadcast over partitions 0:64
      via a ones[64,64] matmul -> elementwise multiply -> out^T store
"""

import os
from contextlib import ExitStack

import numpy as np

import concourse.bass as bass
import concourse.mybir as mybir
import concourse.tile as tile
from concourse import bacc
from concourse.bass_utils import run_bass_kernel_spmd
from concourse.masks import make_identity

B, S, E, H = 8, 2048, 1024, 64
P = 128
QB = 512  # q-block (psum free dim)
F32 = mybir.dt.float32
BF16 = mybir.dt.bfloat16


def build_kernel_body(tc, xp_d, wqk_d, wv_d, out_d, s=S, e_dim=E):
    nc = tc.nc
    EC = e_dim // P  # e-chunks
    NQB = s // QB    # q-blocks
    NT = s // P      # s-tiles of 128
    KPQ = QB // P    # k-chunks per q-block (4)

    ctx = ExitStack()
    with ctx:
        const = ctx.enter_context(tc.tile_pool(name="const", bufs=1))
        big = ctx.enter_context(tc.tile_pool(name="big", bufs=1))

        # weights on the ACT HWDGE ring so the Sync ring starts on x
        wqk_sb = const.tile([P, EC, 2 * H], BF16)
        nc.scalar.dma_start(wqk_sb[:], wqk_d[:])
        wv_sb = const.tile([P, EC, H], BF16)
        nc.scalar.dma_start(wv_sb[:], wv_d[:])
        ident_bf = const.tile([P, P], BF16)
        make_identity(nc, ident_bf[:])
        ones_sb = const.tile([H, H], BF16)
        nc.gpsimd.memset(ones_sb[:], 1.0)
        recb_sb = const.tile([H, QB], BF16)
        nc.gpsimd.memset(recb_sb[:], 0.0)  # rows 1:64 stay zero

        # x blocks q-block-major; first q-block split so the first
        # projection matmul can start after 256 KB
        xp_sb = big.tile([P, EC, s], BF16)
        for qb in range(NQB):
            for g in range(EC // 2):
                nc.sync.dma_start(
                    xp_sb[:, 2 * g:2 * g + 2, qb * QB:(qb + 1) * QB],
                    xp_d[:, 2 * g:2 * g + 2, qb * QB:(qb + 1) * QB])

        qkT_sb = big.tile([P, s], BF16)  # rows 0:64 qT, rows 64:128 kT
        kT_sb = big.tile([H, s], BF16)   # kT at base partition 0
        qT2_sb = big.tile([P, s], BF16)  # qT duplicated at rows 64:128
        vT_sb = big.tile([H, s], BF16)
        v_sb = big.tile([P, NT, H + 1], BF16)  # natural v + ones col
        outT_sb = big.tile([H, s], F32)

        nc.gpsimd.memset(v_sb[:, :, H:H + 1], 1.0)

        # PSUM budget (8 banks): pqk 1 + pvt 1 + ps 3 (also serves the
        # bcast matmul) + po 2 + ptr 1 = 8
        pqk = ctx.enter_context(tc.tile_pool(name="ps_qk", bufs=1, space="PSUM"))
        pvt = ctx.enter_context(tc.tile_pool(name="ps_vt", bufs=1, space="PSUM"))
        ps = ctx.enter_context(tc.tile_pool(name="ps_s", bufs=3, space="PSUM"))
        po = ctx.enter_context(tc.tile_pool(name="ps_o", bufs=2, space="PSUM"))
        ptr = ctx.enter_context(tc.tile_pool(name="ps_tr", bufs=1, space="PSUM"))
        ep = ctx.enter_context(tc.tile_pool(name="expp", bufs=6))
        sp = ctx.enter_context(tc.tile_pool(name="smalls", bufs=4))

        psum_o_pend = [None] * NQB

        # two-stage normalize: the reciprocal (DVE streams the free dim
        # at ~6.5 cyc/elem -- 3.4 us for 512 on one lane) is issued
        # right after its attention block and runs during the NEXT
        # block's attention; the bf16 broadcast matmul + multiply +
        # store run one block later so the PE never waits on the recip.
        def normalize_a(qb):
            psum_o = psum_o_pend[qb]
            # bf16 reciprocals add <1e-4 to the end-to-end rel err
            # (verified against a float64 reference emulation)
            with nc.allow_low_precision(reason="bf16 softmax recip ok"):
                nc.vector.reciprocal(recb_sb[0:1, :], psum_o[H:H + 1, :])

        def normalize_b(qb):
            qsl = slice(qb * QB, (qb + 1) * QB)
            psum_o = psum_o_pend[qb]
            psum_b = ps.tile([P, QB], F32, tag="sc")
            nc.tensor.matmul(
                psum_b[0:H, :], lhsT=ones_sb[:], rhs=recb_sb[:],
                start=True, stop=True)
            bcast = sp.tile([H, QB], F32, tag="bc")
            nc.vector.tensor_copy(bcast[:], psum_b[0:H, :])
            nc.vector.tensor_mul(
                out=outT_sb[:, qsl], in0=psum_o[0:H, :], in1=bcast[:])
            nc.sync.dma_start(out_d[:, qsl], outT_sb[:, qsl])

        for qb in range(NQB):
            qsl = slice(qb * QB, (qb + 1) * QB)
            # ---- projections for this q-block ----
            psum_qk = pqk.tile([P, QB], F32, tag="qk")
            psum_vT = pvt.tile([H, QB], F32, tag="vt")
            for ec in range(EC):
                nc.tensor.matmul(
                    psum_qk[:], lhsT=wqk_sb[:, ec, :],
                    rhs=xp_sb[:, ec, qsl],
                    start=(ec == 0), stop=(ec == EC - 1))
                nc.tensor.matmul(
                    psum_vT[:], lhsT=wv_sb[:, ec, :],
                    rhs=xp_sb[:, ec, qsl],
                    start=(ec == 0), stop=(ec == EC - 1))
            nc.vector.tensor_copy(qkT_sb[:, qsl], psum_qk[:])
            nc.gpsimd.dma_start(kT_sb[:, qsl], qkT_sb[H:P, qsl])
            nc.gpsimd.dma_start(qT2_sb[H:P, qsl], qkT_sb[0:H, qsl])
            nc.vector.tensor_copy(vT_sb[:, qsl], psum_vT[:])
            for t in range(qb * KPQ, (qb + 1) * KPQ):
                pvtr = ptr.tile([P, H], BF16, tag="tr")
                nc.tensor.transpose(
                    pvtr[:], vT_sb[:, t * P:(t + 1) * P], ident_bf[0:H, 0:H])
                nc.vector.tensor_copy(v_sb[:, t, 0:H], pvtr[:])

            # ---- attention for this q-block ----
            nkc = (qb + 1) * KPQ
            psum_o = po.tile([H + 1, QB], F32)
            psum_o_pend[qb] = psum_o
            for pr in range(nkc // 2):
                kc0, kc1 = 2 * pr, 2 * pr + 1
                # row-packed pair: kc0 on PE rows 0:64, kc1 on rows
                # 64:128 (kT lives at rows 64:128 of qkT_sb; qT
                # duplicated there); the two matmuls run concurrently
                psum_prs = []
                offs = []
                for i, kc in enumerate((kc0, kc1)):
                    o = max(0, kc * P - qb * QB)
                    offs.append(o)
                    psum_s = ps.tile([P, QB], F32, tag="sc")
                    psum_prs.append(psum_s)
                    if i == 0:
                        nc.tensor.matmul(
                            psum_s[:, o:],
                            lhsT=kT_sb[:, kc * P:(kc + 1) * P],
                            rhs=qkT_sb[0:H, qsl][:, o:],
                            start=True, stop=True)
                    else:
                        nc.tensor.matmul(
                            psum_s[:, o:],
                            lhsT=qkT_sb[H:P, kc * P:(kc + 1) * P],
                            rhs=qT2_sb[H:P, qsl][:, o:],
                            start=True, stop=True)
                for i, (kc, o) in enumerate(((kc0, offs[0]), (kc1, offs[1]))):
                    et = ep.tile([P, QB], BF16)
                    nc.scalar.activation(
                        et[:, o:], psum_prs[i][:, o:],
                        mybir.ActivationFunctionType.Exp, scale=1.0 / H)
                    if kc * P - qb * QB >= 0:
                        # diagonal chunk: keep where q >= k (j - p >= 0)
                        nc.gpsimd.affine_select(
                            out=et[:, o:], in_=et[:, o:],
                            compare_op=mybir.AluOpType.is_ge,
                            fill=0.0, base=0,
                            channel_multiplier=-1,
                            pattern=[[1, QB - o]])
                    nc.tensor.matmul(
                        psum_o[:, o:],
                        lhsT=v_sb[:, kc, :],
                        rhs=et[:, o:],
                        start=(kc == 0), stop=(kc == nkc - 1))

            # finish the previous block (reads recb before this block's
            # reciprocal overwrites it -- program order carries the WAR)
            if qb > 0:
                normalize_b(qb - 1)
            normalize_a(qb)

        normalize_b(NQB - 1)


def build_bass(s=S, e_dim=E, n_cores=B):
    nc = bacc.Bacc(
        "TRN2", target_bir_lowering=False, debug=False, num_devices=n_cores)
    EC = e_dim // P
    xp_d = nc.dram_tensor("xp", [P, EC, s], BF16, kind="ExternalInput").ap()
    wqk_d = nc.dram_tensor(
        "wqk", [P, EC, 2 * H], BF16, kind="ExternalInput").ap()
    wv_d = nc.dram_tensor("wv", [P, EC, H], BF16, kind="ExternalInput").ap()
    out_d = nc.dram_tensor("out", [H, s], F32, kind="ExternalOutput").ap()
    with tile.TileContext(nc) as tc:
        build_kernel_body(tc, xp_d, wqk_d, wv_d, out_d, s=s, e_dim=e_dim)
    nc.compile()
    return nc


_nc_cache = None


def _ensure_ntff_hook():
    """Dev-only: provide the antenv.axon_hooks shim so trace=True can
    capture NTFF profiles through libaxon_pjrt.so in this container."""
    import sys
    import types
    import ctypes
    import contextlib

    try:
        from antenv.axon_hooks import get_axon_ntff_profile_hook  # noqa
        return
    except ImportError:
        pass
    import antenv

    mod = types.ModuleType("antenv.axon_hooks")
    _h = [None]
    mod.set_axon_ntff_profile_hook = lambda h: _h.__setitem__(0, h)
    mod.get_axon_ntff_profile_hook = lambda: _h[0]
    sys.modules["antenv.axon_hooks"] = mod
    antenv.axon_hooks = mod

    so_path = "/opt/axon/libaxon_pjrt.so"
    lib = ctypes.CDLL(so_path)
    if not hasattr(lib, "axon_start_nrt_profile"):
        return
    lib.axon_start_nrt_profile.argtypes = [
        ctypes.POINTER(ctypes.c_int64), ctypes.c_size_t]
    lib.axon_start_nrt_profile.restype = ctypes.c_int64
    lib.axon_stop_nrt_profile.argtypes = [ctypes.c_char_p]
    lib.axon_stop_nrt_profile.restype = ctypes.c_int64

    @contextlib.contextmanager
    def _hook(output_dir, device_ids):
        import jax
        jax.devices()
        if device_ids:
            ids = (ctypes.c_int64 * len(device_ids))(*device_ids)
            rc = lib.axon_start_nrt_profile(ids, len(device_ids))
        else:
            rc = lib.axon_start_nrt_profile(None, 0)
        if rc != 0:
            raise RuntimeError(f"axon_start_nrt_profile rc={rc}")
        try:
            yield
        finally:
            n = lib.axon_stop_nrt_profile(str(output_dir).encode())
            print(f"profile: {n} file(s) written to {output_dir}")

    mod.set_axon_ntff_profile_hook(_hook)

    # no bucket access in this container; keep artifacts local
    import concourse.bass_utils as bu
    bu.upload_artifacts = lambda tmpdir: tmpdir


def _swizzle(a, ec, p):
    """[E, M] -> [P, EC, M] with [pp, c, m] = a[c*p + pp, m]."""
    return np.ascontiguousarray(a.reshape(ec, p, a.shape[-1]).transpose(1, 0, 2))


def kernel(x, Wq, Wk, Wv):
    global _nc_cache
    import ml_dtypes
    bf = ml_dtypes.bfloat16

    x = np.asarray(x, dtype=np.float32)
    Wq = np.asarray(Wq, dtype=np.float32)
    Wk = np.asarray(Wk, dtype=np.float32)
    Wv = np.asarray(Wv, dtype=np.float32)

    if _nc_cache is None:
        _nc_cache = build_bass()
    nc = _nc_cache

    EC = E // P
    wqk = _swizzle(np.concatenate([Wq, Wk], axis=1).astype(bf), EC, P)
    wv = _swizzle(Wv.astype(bf), EC, P)
    in_maps = []
    for b in range(B):
        in_maps.append({
            "xp": _swizzle(x[b].T.astype(bf), EC, P),
            "wqk": wqk,
            "wv": wv,
        })

    trace = bool(int(os.environ.get("ATTN_TRACE", "0")))
    if trace:
        _ensure_ntff_hook()
    res = run_bass_kernel_spmd(
        nc, in_maps, core_ids=list(range(B)), trace=trace)
    if trace and res.exec_time_ns is not None:
        print(f"HW exec time: {res.exec_time_ns} ns")
        kernel.last_exec_time_ns = res.exec_time_ns
        kernel.last_results = res
    # out^T [64, S] per core -> [B, S, 64]
    out = np.stack(
        [np.ascontiguousarray(res.results[b]["out"].T) for b in range(B)],
        axis=0)
    return out


# revision 28
# speedup vs baseline: 1.1392x; 1.0524x over previous
"""Causal single-head self-attention on 8 Trainium2 NeuronCores.

Problem: x:[8,2048,1024], Wq/Wk/Wv:[1024,64] ->
    out[b] = softmax(tril(x[b]Wq (x[b]Wk)^T / 64)) @ (x[b]Wv)   [8,2048,64]

Sharding: data-parallel over batch -- core b gets batch element b.
Weights replicated.

Per-core algorithm (fp32 matmuls on TRN2 lower to 2x LOW_HIGH PE passes,
so all matmul operands are bf16 with fp32 PSUM accumulation; measured
end-to-end rel err ~3.8e-3 against the fp32 reference):
  - host pre-swizzles x[b] into the exact SBUF layout xp[128, 8, 2048]
    (partition, e-chunk, seq) in bf16 so every DMA is dense, and
    likewise the weights; kernel output is out^T [64, S] fp32,
    un-transposed on the host
  - per q-block of 512 (pipelined with the xT DMA):
      qkT[128, qb]: rows 0:64 = q^T, 64:128 = k^T via packed projection
      (lhsT=[Wq|Wk][e]); kT DMA-shifted to partitions 0:64 and qT
      DMA-shifted up to partitions 64:128 so score matmuls can be
      row-packed two-at-a-time on PE row groups 0:64 / 64:128;
      v^T projected likewise then PE-transposed to natural v[s,64] bf16
      with a ones column appended -> v_aug[s, 65]
      attention: scores^T[kc, qb] = kT_kc.T @ qT_qb (K=64, fp32 psum);
      exp via ACT (scale=1/64, fp32 in, bf16 out); causal = skip
      above-diagonal chunks + truncate diagonal chunks' q-range +
      gpsimd affine_select triangular mask; out^T psum[65, qb] +=
      v_aug[kc].T @ expT; row 64 accumulates the softmax denominators
      normalize: reciprocal of row 64 -> broadcast over partitions 0:64
      via a ones[64,64] matmul -> elementwise multiply -> out^T store
"""

import os
from contextlib import ExitStack

import numpy as np

import concourse.bass as bass
import concourse.mybir as mybir
import concourse.tile as tile
from concourse import bacc
from concourse.bass_utils import run_bass_kernel_spmd
from concourse.masks import make_identity

B, S, E, H = 8, 2048, 1024, 64
P = 128
QB = 512  # q-block (psum free dim)
F32 = mybir.dt.float32
BF16 = mybir.dt.bfloat16


def build_kernel_body(tc, xp_d, wqk_d, wv_d, out_d, s=S, e_dim=E):
    nc = tc.nc
    EC = e_dim // P  # e-chunks
    NQB = s // QB    # q-blocks
    NT = s // P      # s-tiles of 128
    KPQ = QB // P    # k-chunks per q-block (4)

    ctx = ExitStack()
    with ctx:
        const = ctx.enter_context(tc.tile_pool(name="const", bufs=1))
        big = ctx.enter_context(tc.tile_pool(name="big", bufs=1))

        # weights on the ACT HWDGE ring so the Sync ring starts on x
        wqk_sb = const.tile([P, EC, 2 * H], BF16)
        nc.scalar.dma_start(wqk_sb[:], wqk_d[:])
        wv_sb = const.tile([P, EC, H], BF16)
        nc.scalar.dma_start(wv_sb[:], wv_d[:])
        ident_bf = const.tile([P, P], BF16)
        make_identity(nc, ident_bf[:])
        ones_sb = const.tile([H, H], BF16)
        nc.gpsimd.memset(ones_sb[:], 1.0)
        recb_sb = const.tile([H, QB], BF16)
        nc.gpsimd.memset(recb_sb[:], 0.0)  # rows 1:64 stay zero

        # x blocks q-block-major; first q-block split so the first
        # projection matmul can start after 256 KB
        xp_sb = big.tile([P, EC, s], BF16)
        for qb in range(NQB):
            for g in range(EC // 2):
                nc.sync.dma_start(
                    xp_sb[:, 2 * g:2 * g + 2, qb * QB:(qb + 1) * QB],
                    xp_d[:, 2 * g:2 * g + 2, qb * QB:(qb + 1) * QB])

        qkT_sb = big.tile([P, s], BF16)  # rows 0:64 qT, rows 64:128 kT
        kT_sb = big.tile([H, s], BF16)   # kT at base partition 0
        qT2_sb = big.tile([P, s], BF16)  # qT duplicated at rows 64:128
        vT_sb = big.tile([H, s], BF16)
        v_sb = big.tile([P, NT, H + 1], BF16)  # natural v + ones col
        outT_sb = big.tile([H, s], F32)

        nc.gpsimd.memset(v_sb[:, :, H:H + 1], 1.0)

        # PSUM budget (8 banks): pqk 1 + pvt 1 + ps 3 (also serves the
        # bcast matmul) + po 2 + ptr 1 = 8
        pqk = ctx.enter_context(tc.tile_pool(name="ps_qk", bufs=1, space="PSUM"))
        pvt = ctx.enter_context(tc.tile_pool(name="ps_vt", bufs=1, space="PSUM"))
        ps = ctx.enter_context(tc.tile_pool(name="ps_s", bufs=3, space="PSUM"))
        po = ctx.enter_context(tc.tile_pool(name="ps_o", bufs=2, space="PSUM"))
        ptr = ctx.enter_context(tc.tile_pool(name="ps_tr", bufs=1, space="PSUM"))
        ep = ctx.enter_context(tc.tile_pool(name="expp", bufs=6))
        sp = ctx.enter_context(tc.tile_pool(name="smalls", bufs=4))

        psum_o_pend = [None] * NQB

        # two-stage normalize: the reciprocal (DVE streams the free dim
        # at ~6.5 cyc/elem -- 3.4 us for 512 on one lane) is issued
        # right after its attention block and runs during the NEXT
        # block's attention; the bf16 broadcast matmul + multiply +
        # store run one block later so the PE never waits on the recip.
        def normalize_a(qb):
            psum_o = psum_o_pend[qb]
            # bf16 reciprocals add <1e-4 to the end-to-end rel err
            # (verified against a float64 reference emulation)
            with nc.allow_low_precision(reason="bf16 softmax recip ok"):
                nc.vector.reciprocal(recb_sb[0:1, :], psum_o[H:H + 1, :])

        def normalize_b(qb):
            qsl = slice(qb * QB, (qb + 1) * QB)
            psum_o = psum_o_pend[qb]
            psum_b = ps.tile([P, QB], F32, tag="sc")
            nc.tensor.matmul(
                psum_b[0:H, :], lhsT=ones_sb[:], rhs=recb_sb[:],
                start=True, stop=True)
            bcast = sp.tile([H, QB], F32, tag="bc")
            nc.vector.tensor_copy(bcast[:], psum_b[0:H, :])
            nc.vector.tensor_mul(
                out=outT_sb[:, qsl], in0=psum_o[0:H, :], in1=bcast[:])
            nc.sync.dma_start(out_d[:, qsl], outT_sb[:, qsl])

        def proj_items(qb):
            """Projection work for q-block qb as a list of emit-callbacks
            (PE items that can fill exp-latency bubbles in the previous
            q-block's attention stream)."""
            qsl = slice(qb * QB, (qb + 1) * QB)
            psum_qk = pqk.tile([P, QB], F32, tag="qk")
            psum_vT = pvt.tile([H, QB], F32, tag="vt")
            items = []

            def mk_mm(ec):
                def f():
                    nc.tensor.matmul(
                        psum_qk[:], lhsT=wqk_sb[:, ec, :],
                        rhs=xp_sb[:, ec, qsl],
                        start=(ec == 0), stop=(ec == EC - 1))
                    nc.tensor.matmul(
                        psum_vT[:], lhsT=wv_sb[:, ec, :],
                        rhs=xp_sb[:, ec, qsl],
                        start=(ec == 0), stop=(ec == EC - 1))
                return f
            for ec in range(EC):
                items.append(mk_mm(ec))

            def copies():
                nc.vector.tensor_copy(qkT_sb[:, qsl], psum_qk[:])
                nc.gpsimd.dma_start(kT_sb[:, qsl], qkT_sb[H:P, qsl])
                nc.gpsimd.dma_start(qT2_sb[H:P, qsl], qkT_sb[0:H, qsl])
                nc.vector.tensor_copy(vT_sb[:, qsl], psum_vT[:])
            items.append(copies)

            def mk_vtr(t):
                def f():
                    pvtr = ptr.tile([P, H], BF16, tag="tr")
                    nc.tensor.transpose(
                        pvtr[:], vT_sb[:, t * P:(t + 1) * P],
                        ident_bf[0:H, 0:H])
                    nc.vector.tensor_copy(v_sb[:, t, 0:H], pvtr[:])
                return f
            for t in range(qb * KPQ, (qb + 1) * KPQ):
                items.append(mk_vtr(t))
            return items

        def emit_attention(qb, filler):
            """Attention for q-block qb; `filler` items are emitted
            between each score pair and its attn@v pair so the PE has
            independent work while ACT computes the exp."""
            qsl = slice(qb * QB, (qb + 1) * QB)
            nkc = (qb + 1) * KPQ
            npairs = nkc // 2
            psum_o = po.tile([H + 1, QB], F32)
            psum_o_pend[qb] = psum_o
            fill_idx = 0
            for pr in range(npairs):
                kc0, kc1 = 2 * pr, 2 * pr + 1
                # row-packed pair: kc0 on PE rows 0:64, kc1 on rows
                # 64:128 (kT lives at rows 64:128 of qkT_sb; qT
                # duplicated there); the two matmuls run concurrently
                pair = []
                for i, kc in enumerate((kc0, kc1)):
                    o = max(0, kc * P - qb * QB)
                    psum_s = ps.tile([P, QB], F32, tag="sc")
                    pair.append((kc, o, psum_s))
                    if i == 0:
                        nc.tensor.matmul(
                            psum_s[:, o:],
                            lhsT=kT_sb[:, kc * P:(kc + 1) * P],
                            rhs=qkT_sb[0:H, qsl][:, o:],
                            start=True, stop=True)
                    else:
                        nc.tensor.matmul(
                            psum_s[:, o:],
                            lhsT=qkT_sb[H:P, kc * P:(kc + 1) * P],
                            rhs=qT2_sb[H:P, qsl][:, o:],
                            start=True, stop=True)
                ets = []
                for kc, o, psum_s in pair:
                    et = ep.tile([P, QB], BF16)
                    nc.scalar.activation(
                        et[:, o:], psum_s[:, o:],
                        mybir.ActivationFunctionType.Exp, scale=1.0 / H)
                    if kc * P - qb * QB >= 0:
                        # diagonal chunk: keep where q >= k (j - p >= 0)
                        nc.gpsimd.affine_select(
                            out=et[:, o:], in_=et[:, o:],
                            compare_op=mybir.AluOpType.is_ge,
                            fill=0.0, base=0,
                            channel_multiplier=-1,
                            pattern=[[1, QB - o]])
                    ets.append((kc, o, et))
                # fill the exp latency with next-block projection work
                take = ((pr + 1) * len(filler)) // npairs - fill_idx
                for _ in range(take):
                    filler[fill_idx]()
                    fill_idx += 1
                for kc, o, et in ets:
                    nc.tensor.matmul(
                        psum_o[:, o:],
                        lhsT=v_sb[:, kc, :],
                        rhs=et[:, o:],
                        start=(kc == 0), stop=(kc == nkc - 1))
            while fill_idx < len(filler):
                filler[fill_idx]()
                fill_idx += 1

        # q-block 0's projection is DMA-paced; emit it plainly
        for it in proj_items(0):
            it()
        for qb in range(NQB):
            filler = proj_items(qb + 1) if qb + 1 < NQB else []
            emit_attention(qb, filler)
            # finish the previous block (reads recb before this block's
            # reciprocal overwrites it -- program order carries the WAR)
            if qb > 0:
                normalize_b(qb - 1)
            normalize_a(qb)

        normalize_b(NQB - 1)


def build_bass(s=S, e_dim=E, n_cores=B):
    nc = bacc.Bacc(
        "TRN2", target_bir_lowering=False, debug=False, num_devices=n_cores)
    EC = e_dim // P
    xp_d = nc.dram_tensor("xp", [P, EC, s], BF16, kind="ExternalInput").ap()
    wqk_d = nc.dram_tensor(
        "wqk", [P, EC, 2 * H], BF16, kind="ExternalInput").ap()
    wv_d = nc.dram_tensor("wv", [P, EC, H], BF16, kind="ExternalInput").ap()
    out_d = nc.dram_tensor("out", [H, s], F32, kind="ExternalOutput").ap()
    with tile.TileContext(nc) as tc:
        build_kernel_body(tc, xp_d, wqk_d, wv_d, out_d, s=s, e_dim=e_dim)
    nc.compile()
    return nc


_nc_cache = None


def _ensure_ntff_hook():
    """Dev-only: provide the antenv.axon_hooks shim so trace=True can
    capture NTFF profiles through libaxon_pjrt.so in this container."""
    import sys
    import types
    import ctypes
    import contextlib

    try:
        from antenv.axon_hooks import get_axon_ntff_profile_hook  # noqa
        return
    except ImportError:
        pass
    import antenv

    mod = types.ModuleType("antenv.axon_hooks")
    _h = [None]
    mod.set_axon_ntff_profile_hook = lambda h: _h.__setitem__(0, h)
    mod.get_axon_ntff_profile_hook = lambda: _h[0]
    sys.modules["antenv.axon_hooks"] = mod
    antenv.axon_hooks = mod

    so_path = "/opt/axon/libaxon_pjrt.so"
    lib = ctypes.CDLL(so_path)
    if not hasattr(lib, "axon_start_nrt_profile"):
        return
    lib.axon_start_nrt_profile.argtypes = [
        ctypes.POINTER(ctypes.c_int64), ctypes.c_size_t]
    lib.axon_start_nrt_profile.restype = ctypes.c_int64
    lib.axon_stop_nrt_profile.argtypes = [ctypes.c_char_p]
    lib.axon_stop_nrt_profile.restype = ctypes.c_int64

    @contextlib.contextmanager
    def _hook(output_dir, device_ids):
        import jax
        jax.devices()
        if device_ids:
            ids = (ctypes.c_int64 * len(device_ids))(*device_ids)
            rc = lib.axon_start_nrt_profile(ids, len(device_ids))
        else:
            rc = lib.axon_start_nrt_profile(None, 0)
        if rc != 0:
            raise RuntimeError(f"axon_start_nrt_profile rc={rc}")
        try:
            yield
        finally:
            n = lib.axon_stop_nrt_profile(str(output_dir).encode())
            print(f"profile: {n} file(s) written to {output_dir}")

    mod.set_axon_ntff_profile_hook(_hook)

    # no bucket access in this container; keep artifacts local
    import concourse.bass_utils as bu
    bu.upload_artifacts = lambda tmpdir: tmpdir


def _swizzle(a, ec, p):
    """[E, M] -> [P, EC, M] with [pp, c, m] = a[c*p + pp, m]."""
    return np.ascontiguousarray(a.reshape(ec, p, a.shape[-1]).transpose(1, 0, 2))


def kernel(x, Wq, Wk, Wv):
    global _nc_cache
    import ml_dtypes
    bf = ml_dtypes.bfloat16

    x = np.asarray(x, dtype=np.float32)
    Wq = np.asarray(Wq, dtype=np.float32)
    Wk = np.asarray(Wk, dtype=np.float32)
    Wv = np.asarray(Wv, dtype=np.float32)

    if _nc_cache is None:
        _nc_cache = build_bass()
    nc = _nc_cache

    EC = E // P
    wqk = _swizzle(np.concatenate([Wq, Wk], axis=1).astype(bf), EC, P)
    wv = _swizzle(Wv.astype(bf), EC, P)
    in_maps = []
    for b in range(B):
        in_maps.append({
            "xp": _swizzle(x[b].T.astype(bf), EC, P),
            "wqk": wqk,
            "wv": wv,
        })

    trace = bool(int(os.environ.get("ATTN_TRACE", "0")))
    if trace:
        _ensure_ntff_hook()
    res = run_bass_kernel_spmd(
        nc, in_maps, core_ids=list(range(B)), trace=trace)
    if trace and res.exec_time_ns is not None:
        print(f"HW exec time: {res.exec_time_ns} ns")
        kernel.last_exec_time_ns = res.exec_time_ns
        kernel.last_results = res
    # out^T [64, S] per core -> [B, S, 64]
    out = np.stack(
        [np.ascontiguousarray(res.results[b]["out"].T) for b in range(B)],
        axis=0)
    return out


# revision 34
# speedup vs baseline: 1.4152x; 1.2423x over previous
"""Causal single-head self-attention on 8 Trainium2 NeuronCores.

Problem: x:[8,2048,1024], Wq/Wk/Wv:[1024,64] ->
    out[b] = softmax(tril(x[b]Wq (x[b]Wk)^T / 64)) @ (x[b]Wv)   [8,2048,64]

Sharding: data-parallel over batch -- core b gets batch element b.
Weights replicated.

Per-core algorithm (fp32 matmuls on TRN2 lower to 2x LOW_HIGH PE passes,
so all matmul operands are bf16 with fp32 PSUM accumulation; measured
end-to-end rel err ~3.8e-3 against the fp32 reference):
  - host pre-swizzles x[b] into the exact SBUF layout xp[128, 8, 2048]
    (partition, e-chunk, seq) in bf16 so every DMA is dense, and
    likewise the weights; kernel output is out^T [64, S] fp32,
    un-transposed on the host
  - per q-block of 512 (pipelined with the xT DMA):
      qkT[128, qb]: rows 0:64 = q^T, 64:128 = k^T via packed projection
      (lhsT=[Wq|Wk][e]); kT DMA-shifted to partitions 0:64 and qT
      DMA-shifted up to partitions 64:128 so score matmuls can be
      row-packed two-at-a-time on PE row groups 0:64 / 64:128;
      v^T projected likewise then PE-transposed to natural v[s,64] bf16
      with a ones column appended -> v_aug[s, 65]
      attention: scores^T[kc, qb] = kT_kc.T @ qT_qb (K=64, fp32 psum);
      exp via ACT (scale=1/64, fp32 in, bf16 out); causal = skip
      above-diagonal chunks + truncate diagonal chunks' q-range +
      gpsimd affine_select triangular mask; out^T psum[65, qb] +=
      v_aug[kc].T @ expT; row 64 accumulates the softmax denominators
      normalize: reciprocal of row 64 -> broadcast over partitions 0:64
      via a ones[64,64] matmul -> elementwise multiply -> out^T store
"""

import os
from contextlib import ExitStack

import numpy as np

import concourse.bass as bass
import concourse.mybir as mybir
import concourse.tile as tile
from concourse import bacc
from concourse.bass_utils import run_bass_kernel_spmd
from concourse.masks import make_identity

B, S, E, H = 8, 2048, 1024, 64
P = 128
QB = 512  # q-block (psum free dim)
F32 = mybir.dt.float32
BF16 = mybir.dt.bfloat16


def build_kernel_body(tc, xp_d, wqk_d, wv_d, out_d, s=S, e_dim=E):
    nc = tc.nc
    EC = e_dim // P  # e-chunks
    NQB = s // QB    # q-blocks
    NT = s // P      # s-tiles of 128
    KPQ = QB // P    # k-chunks per q-block (4)

    ctx = ExitStack()
    with ctx:
        const = ctx.enter_context(tc.tile_pool(name="const", bufs=1))
        big = ctx.enter_context(tc.tile_pool(name="big", bufs=1))

        # weights on the ACT HWDGE ring so the Sync ring starts on x
        wqk_sb = const.tile([P, EC, 2 * H], BF16)
        nc.scalar.dma_start(wqk_sb[:], wqk_d[:])
        wv_sb = const.tile([P, EC, H], BF16)
        nc.scalar.dma_start(wv_sb[:], wv_d[:])
        ones_sb = const.tile([H, H], BF16)
        nc.gpsimd.memset(ones_sb[:], 1.0)
        recb_sb = const.tile([H, QB], BF16)
        nc.gpsimd.memset(recb_sb[:], 0.0)  # rows 1:64 stay zero

        # x blocks q-block-major; first q-block split so the first
        # projection matmul can start after 256 KB
        xp_sb = big.tile([P, EC, s], BF16)
        for qb in range(NQB):
            for g in range(EC // 2):
                nc.sync.dma_start(
                    xp_sb[:, 2 * g:2 * g + 2, qb * QB:(qb + 1) * QB],
                    xp_d[:, 2 * g:2 * g + 2, qb * QB:(qb + 1) * QB])

        qkT_sb = big.tile([P, s], BF16)  # rows 0:64 qT, rows 64:128 kT
        kT_sb = big.tile([H, s], BF16)   # kT at base partition 0
        qT2_sb = big.tile([P, s], BF16)  # qT duplicated at rows 64:128
        v_sb = big.tile([P, NT, H + 1], BF16)  # natural v + ones col
        outT_sb = big.tile([H, s], F32)

        nc.gpsimd.memset(v_sb[:, :, H:H + 1], 1.0)

        # PSUM budget (8 banks): pqk 1 + pv 1 + ps 2x2 (score pairs;
        # also serves the bcast matmul) + po 2 = 8
        pqk = ctx.enter_context(tc.tile_pool(name="ps_qk", bufs=1, space="PSUM"))
        pv = ctx.enter_context(tc.tile_pool(name="ps_v", bufs=1, space="PSUM"))
        ps = ctx.enter_context(tc.tile_pool(name="ps_s", bufs=2, space="PSUM"))
        po = ctx.enter_context(tc.tile_pool(name="ps_o", bufs=2, space="PSUM"))
        ep = ctx.enter_context(tc.tile_pool(name="expp", bufs=4))
        sp = ctx.enter_context(tc.tile_pool(name="smalls", bufs=4))

        psum_o_pend = [None] * NQB

        # two-stage normalize: the reciprocal (DVE streams the free dim
        # at ~6.5 cyc/elem -- 3.4 us for 512 on one lane) is issued
        # right after its attention block and runs during the NEXT
        # block's attention; the bf16 broadcast matmul + multiply +
        # store run one block later so the PE never waits on the recip.
        def normalize_a(qb):
            psum_o = psum_o_pend[qb]
            # bf16 reciprocals add <1e-4 to the end-to-end rel err
            # (verified against a float64 reference emulation)
            with nc.allow_low_precision(reason="bf16 softmax recip ok"):
                nc.vector.reciprocal(recb_sb[0:1, :], psum_o[H:H + 1, :])

        def normalize_b(qb):
            qsl = slice(qb * QB, (qb + 1) * QB)
            psum_o = psum_o_pend[qb]
            psum_b = ps.tile([P, 2, QB], F32, tag="sc")
            nc.tensor.matmul(
                psum_b[0:H, 0, :], lhsT=ones_sb[:], rhs=recb_sb[:],
                start=True, stop=True)
            bcast = sp.tile([H, QB], F32, tag="bc")
            nc.vector.tensor_copy(bcast[:], psum_b[0:H, 0, :])
            nc.vector.tensor_mul(
                out=outT_sb[:, qsl], in0=psum_o[0:H, :], in1=bcast[:])
            nc.sync.dma_start(out_d[:, qsl], outT_sb[:, qsl])

        def proj_items(qb):
            """Projection work for q-block qb as a list of emit-callbacks
            (PE items that can fill exp-latency bubbles in the previous
            q-block's attention stream)."""
            qsl = slice(qb * QB, (qb + 1) * QB)
            psum_qk = pqk.tile([P, QB], F32, tag="qk")
            items = []

            def mk_mm(ec):
                def f():
                    nc.tensor.matmul(
                        psum_qk[:], lhsT=wqk_sb[:, ec, :],
                        rhs=xp_sb[:, ec, qsl],
                        start=(ec == 0), stop=(ec == EC - 1))
                return f
            for ec in range(EC):
                items.append(mk_mm(ec))

            def copies():
                nc.vector.tensor_copy(qkT_sb[:, qsl], psum_qk[:])
                nc.gpsimd.dma_start(kT_sb[:, qsl], qkT_sb[H:P, qsl])
                nc.gpsimd.dma_start(qT2_sb[H:P, qsl], qkT_sb[0:H, qsl])
            items.append(copies)

            # v in natural layout directly: psum_v[s-tile, 64] +=
            # xp[e, s-tile].T @ Wv[e]  (lhsT = x block, small-N matmuls)
            def mk_v(t, e0, psum_v):
                def f():
                    for ec in range(e0, e0 + EC // 2):
                        nc.tensor.matmul(
                            psum_v[:],
                            lhsT=xp_sb[:, ec, t * P:(t + 1) * P],
                            rhs=wv_sb[:, ec, :],
                            start=(ec == 0), stop=(ec == EC - 1))
                    if e0 + EC // 2 == EC:
                        nc.vector.tensor_copy(v_sb[:, t, 0:H], psum_v[:])
                return f
            for t in range(qb * KPQ, (qb + 1) * KPQ):
                psum_v = pv.tile([P, H], F32, tag="v", name=f"psv{t}")
                items.append(mk_v(t, 0, psum_v))
                items.append(mk_v(t, EC // 2, psum_v))
            return items

        def emit_attention(qb, filler):
            """Attention for q-block qb; `filler` items are emitted
            between each score pair and its attn@v pair so the PE has
            independent work while ACT computes the exp."""
            qsl = slice(qb * QB, (qb + 1) * QB)
            nkc = (qb + 1) * KPQ
            npairs = nkc // 2
            psum_o = po.tile([H + 1, QB], F32)
            psum_o_pend[qb] = psum_o
            fill_idx = 0
            for pr in range(npairs):
                kc0, kc1 = 2 * pr, 2 * pr + 1
                # row-packed pair: kc0 on PE rows 0:64, kc1 on rows
                # 64:128 (kT lives at rows 64:128 of qkT_sb; qT
                # duplicated there); the two matmuls run concurrently
                o0 = max(0, kc0 * P - qb * QB)
                o1 = max(0, kc1 * P - qb * QB)
                psum_pr = ps.tile([P, 2, QB], F32, tag="sc")
                nc.tensor.matmul(
                    psum_pr[:, 0, o0:],
                    lhsT=kT_sb[:, kc0 * P:(kc0 + 1) * P],
                    rhs=qkT_sb[0:H, qsl][:, o0:],
                    start=True, stop=True)
                nc.tensor.matmul(
                    psum_pr[:, 1, o1:],
                    lhsT=qkT_sb[H:P, kc1 * P:(kc1 + 1) * P],
                    rhs=qT2_sb[H:P, qsl][:, o1:],
                    start=True, stop=True)
                et = ep.tile([P, 2, QB], BF16)
                if o0 == o1:
                    # off-diagonal pair: one exp over both banks
                    nc.scalar.activation(
                        et[:, :, o0:], psum_pr[:, :, o0:],
                        mybir.ActivationFunctionType.Exp, scale=1.0 / H)
                else:
                    for i, o in ((0, o0), (1, o1)):
                        nc.scalar.activation(
                            et[:, i, o:], psum_pr[:, i, o:],
                            mybir.ActivationFunctionType.Exp, scale=1.0 / H)
                ets = []
                for i, (kc, o) in enumerate(((kc0, o0), (kc1, o1))):
                    if kc * P - qb * QB >= 0:
                        # diagonal chunk: keep where q >= k (j - p >= 0)
                        nc.gpsimd.affine_select(
                            out=et[:, i, o:], in_=et[:, i, o:],
                            compare_op=mybir.AluOpType.is_ge,
                            fill=0.0, base=0,
                            channel_multiplier=-1,
                            pattern=[[1, QB - o]])
                    ets.append((kc, o, et[:, i, :]))
                # fill the exp latency with next-block projection work
                take = ((pr + 1) * len(filler)) // npairs - fill_idx
                for _ in range(take):
                    filler[fill_idx]()
                    fill_idx += 1
                for kc, o, etv in ets:
                    nc.tensor.matmul(
                        psum_o[:, o:],
                        lhsT=v_sb[:, kc, :],
                        rhs=etv[:, o:],
                        start=(kc == 0), stop=(kc == nkc - 1))
            while fill_idx < len(filler):
                filler[fill_idx]()
                fill_idx += 1

        # q-block 0's projection is DMA-paced; emit it plainly
        for it in proj_items(0):
            it()
        for qb in range(NQB):
            filler = proj_items(qb + 1) if qb + 1 < NQB else []
            emit_attention(qb, filler)
            # finish the previous block (reads recb before this block's
            # reciprocal overwrites it -- program order carries the WAR)
            if qb > 0:
                normalize_b(qb - 1)
            normalize_a(qb)

        normalize_b(NQB - 1)


def build_bass(s=S, e_dim=E, n_cores=B):
    nc = bacc.Bacc(
        "TRN2", target_bir_lowering=False, debug=False, num_devices=n_cores)
    EC = e_dim // P
    xp_d = nc.dram_tensor("xp", [P, EC, s], BF16, kind="ExternalInput").ap()
    wqk_d = nc.dram_tensor(
        "wqk", [P, EC, 2 * H], BF16, kind="ExternalInput").ap()
    wv_d = nc.dram_tensor("wv", [P, EC, H], BF16, kind="ExternalInput").ap()
    out_d = nc.dram_tensor("out", [H, s], F32, kind="ExternalOutput").ap()
    with tile.TileContext(nc) as tc:
        build_kernel_body(tc, xp_d, wqk_d, wv_d, out_d, s=s, e_dim=e_dim)
    nc.compile()
    return nc


_nc_cache = None


def _ensure_ntff_hook():
    """Dev-only: provide the antenv.axon_hooks shim so trace=True can
    capture NTFF profiles through libaxon_pjrt.so in this container."""
    import sys
    import types
    import ctypes
    import contextlib

    try:
        from antenv.axon_hooks import get_axon_ntff_profile_hook  # noqa
        return
    except ImportError:
        pass
    import antenv

    mod = types.ModuleType("antenv.axon_hooks")
    _h = [None]
    mod.set_axon_ntff_profile_hook = lambda h: _h.__setitem__(0, h)
    mod.get_axon_ntff_profile_hook = lambda: _h[0]
    sys.modules["antenv.axon_hooks"] = mod
    antenv.axon_hooks = mod

    so_path = "/opt/axon/libaxon_pjrt.so"
    lib = ctypes.CDLL(so_path)
    if not hasattr(lib, "axon_start_nrt_profile"):
        return
    lib.axon_start_nrt_profile.argtypes = [
        ctypes.POINTER(ctypes.c_int64), ctypes.c_size_t]
    lib.axon_start_nrt_profile.restype = ctypes.c_int64
    lib.axon_stop_nrt_profile.argtypes = [ctypes.c_char_p]
    lib.axon_stop_nrt_profile.restype = ctypes.c_int64

    @contextlib.contextmanager
    def _hook(output_dir, device_ids):
        import jax
        jax.devices()
        if device_ids:
            ids = (ctypes.c_int64 * len(device_ids))(*device_ids)
            rc = lib.axon_start_nrt_profile(ids, len(device_ids))
        else:
            rc = lib.axon_start_nrt_profile(None, 0)
        if rc != 0:
            raise RuntimeError(f"axon_start_nrt_profile rc={rc}")
        try:
            yield
        finally:
            n = lib.axon_stop_nrt_profile(str(output_dir).encode())
            print(f"profile: {n} file(s) written to {output_dir}")

    mod.set_axon_ntff_profile_hook(_hook)

    # no bucket access in this container; keep artifacts local
    import concourse.bass_utils as bu
    bu.upload_artifacts = lambda tmpdir: tmpdir


def _swizzle(a, ec, p):
    """[E, M] -> [P, EC, M] with [pp, c, m] = a[c*p + pp, m]."""
    return np.ascontiguousarray(a.reshape(ec, p, a.shape[-1]).transpose(1, 0, 2))


def kernel(x, Wq, Wk, Wv):
    global _nc_cache
    import ml_dtypes
    bf = ml_dtypes.bfloat16

    x = np.asarray(x, dtype=np.float32)
    Wq = np.asarray(Wq, dtype=np.float32)
    Wk = np.asarray(Wk, dtype=np.float32)
    Wv = np.asarray(Wv, dtype=np.float32)

    if _nc_cache is None:
        _nc_cache = build_bass()
    nc = _nc_cache

    EC = E // P
    wqk = _swizzle(np.concatenate([Wq, Wk], axis=1).astype(bf), EC, P)
    wv = _swizzle(Wv.astype(bf), EC, P)
    in_maps = []
    for b in range(B):
        in_maps.append({
            "xp": _swizzle(x[b].T.astype(bf), EC, P),
            "wqk": wqk,
            "wv": wv,
        })

    trace = bool(int(os.environ.get("ATTN_TRACE", "0")))
    if trace:
        _ensure_ntff_hook()
    res = run_bass_kernel_spmd(
        nc, in_maps, core_ids=list(range(B)), trace=trace)
    if trace and res.exec_time_ns is not None:
        print(f"HW exec time: {res.exec_time_ns} ns")
        kernel.last_exec_time_ns = res.exec_time_ns
        kernel.last_results = res
    # out^T [64, S] per core -> [B, S, 64]
    out = np.stack(
        [np.ascontiguousarray(res.results[b]["out"].T) for b in range(B)],
        axis=0)
    return out


# revision 38
# speedup vs baseline: 1.6101x; 1.1377x over previous
"""Causal single-head self-attention on 8 Trainium2 NeuronCores.

Problem: x:[8,2048,1024], Wq/Wk/Wv:[1024,64] ->
    out[b] = softmax(tril(x[b]Wq (x[b]Wk)^T / 64)) @ (x[b]Wv)   [8,2048,64]

Sharding: data-parallel over batch -- core b gets batch element b.
Weights replicated.

Per-core algorithm (fp32 matmuls on TRN2 lower to 2x LOW_HIGH PE passes,
so all matmul operands are bf16 with fp32 PSUM accumulation; measured
end-to-end rel err ~3.8e-3 against the fp32 reference):
  - host pre-swizzles x[b] into the exact SBUF layout xp[128, 8, 2048]
    (partition, e-chunk, seq) in bf16 so every DMA is dense, and
    likewise the weights; kernel output is out^T [64, S] fp32,
    un-transposed on the host
  - per q-block of 512 (pipelined with the xT DMA):
      qkT[128, qb]: rows 0:64 = q^T, 64:128 = k^T via packed projection
      (lhsT=[Wq|Wk][e]); kT DMA-shifted to partitions 0:64 and qT
      DMA-shifted up to partitions 64:128 so score matmuls can be
      row-packed two-at-a-time on PE row groups 0:64 / 64:128;
      v^T projected likewise then PE-transposed to natural v[s,64] bf16
      with a ones column appended -> v_aug[s, 65]
      attention: scores^T[kc, qb] = kT_kc.T @ qT_qb (K=64, fp32 psum);
      exp via ACT (scale=1/64, fp32 in, bf16 out); causal = skip
      above-diagonal chunks + truncate diagonal chunks' q-range +
      gpsimd affine_select triangular mask; out^T psum[65, qb] +=
      v_aug[kc].T @ expT; row 64 accumulates the softmax denominators
      normalize: reciprocal of row 64 -> broadcast over partitions 0:64
      via a ones[64,64] matmul -> elementwise multiply -> out^T store
"""

import os
from contextlib import ExitStack

import numpy as np

import concourse.bass as bass
import concourse.mybir as mybir
import concourse.tile as tile
from concourse import bacc
from concourse.bass_utils import run_bass_kernel_spmd
from concourse.masks import make_identity

B, S, E, H = 8, 2048, 1024, 64
P = 128
QB = 512  # q-block (psum free dim)
F32 = mybir.dt.float32
BF16 = mybir.dt.bfloat16


def build_kernel_body(tc, xp_d, wqk_d, wkq_d, wv_d, out_d, s=S, e_dim=E):
    nc = tc.nc
    EC = e_dim // P  # e-chunks
    NQB = s // QB    # q-blocks
    NT = s // P      # s-tiles of 128
    KPQ = QB // P    # k-chunks per q-block (4)

    ctx = ExitStack()
    with ctx:
        const = ctx.enter_context(tc.tile_pool(name="const", bufs=1))
        big = ctx.enter_context(tc.tile_pool(name="big", bufs=1))

        # weights on the ACT HWDGE ring so the Sync ring starts on x
        wqk_sb = const.tile([P, EC, 2 * H], BF16)
        nc.scalar.dma_start(wqk_sb[:], wqk_d[:])
        wkq_sb = const.tile([P, EC, 2 * H], BF16)
        nc.scalar.dma_start(wkq_sb[:], wkq_d[:])
        wv_sb = const.tile([P, EC, H], BF16)
        nc.scalar.dma_start(wv_sb[:], wv_d[:])
        ident32 = const.tile([H + 1, H + 1], F32)
        make_identity(nc, ident32[:])

        # x blocks q-block-major; first q-block split so the first
        # projection matmul can start after 256 KB
        xp_sb = big.tile([P, EC, s], BF16)
        for qb in range(NQB):
            for g in range(EC // 2):
                nc.sync.dma_start(
                    xp_sb[:, 2 * g:2 * g + 2, qb * QB:(qb + 1) * QB],
                    xp_d[:, 2 * g:2 * g + 2, qb * QB:(qb + 1) * QB])

        qkT_sb = big.tile([P, s], BF16)  # rows 0:64 qT, rows 64:128 kT
        kq2_sb = big.tile([P, s], BF16)  # rows 0:64 kT, rows 64:128 qT
        v_sb = big.tile([P, NT, H + 1], BF16)  # natural v + ones col
        out_sb = big.tile([P, NT, H], F32)

        nc.gpsimd.memset(v_sb[:, :, H:H + 1], 1.0)

        # PSUM budget (8 banks): pqk 1 + pv 1 + ps 2x2 (score pairs)
        # + po 1 + ptr 1 = 8
        pqk = ctx.enter_context(tc.tile_pool(name="ps_qk", bufs=1, space="PSUM"))
        pv = ctx.enter_context(tc.tile_pool(name="ps_v", bufs=1, space="PSUM"))
        ps = ctx.enter_context(tc.tile_pool(name="ps_s", bufs=2, space="PSUM"))
        po = ctx.enter_context(tc.tile_pool(name="ps_o", bufs=1, space="PSUM"))
        ptr = ctx.enter_context(tc.tile_pool(name="ps_tr", bufs=1, space="PSUM"))
        ep = ctx.enter_context(tc.tile_pool(name="expp", bufs=4))
        sp = ctx.enter_context(tc.tile_pool(name="smalls", bufs=4))

        psum_o_pend = [None] * NQB

        def normalize_items(qb):
            """Normalize q-block qb as interleavable emit-items:
            out^T psum[65, QB] -> copy to SBUF, PE-transpose each
            [65,128] slice to [128,65] (col 64 = denominators),
            per-partition reciprocal + scale, natural-layout store.
            The transposes alternate between two psum pools so each can
            issue while the DVE consumes the previous one."""
            psum_o = psum_o_pend[qb]
            oT = sp.tile([H + 1, QB], F32, tag="oT")
            items = [lambda: nc.vector.tensor_copy(oT[:], psum_o[:])]

            def mk_j(j):
                def f():
                    if j % 2 == 0:
                        pt2 = ptr.tile([P, H + 1], F32, tag="tr")
                    else:
                        pt2 = ps.tile([P, 2, QB], F32, tag="sc", name="pt2s")[:, 0, 0:H + 1]
                    nc.tensor.transpose(
                        pt2[:], oT[:, j * P:(j + 1) * P], ident32[:])
                    rec = sp.tile([P, 1], F32, tag="rec")
                    nc.vector.reciprocal(rec[:], pt2[:, H:H + 1])
                    t = qb * KPQ + j
                    nc.vector.tensor_scalar_mul(
                        out_sb[:, t, :], pt2[:, 0:H], rec[:])
                return f
            for j in range(KPQ):
                items.append(mk_j(j))
            items.append(lambda: nc.sync.dma_start(
                out_d[:, qb * KPQ:(qb + 1) * KPQ, :],
                out_sb[:, qb * KPQ:(qb + 1) * KPQ, :]))
            return items

        def proj_items(qb):
            """Projection work for q-block qb as a list of emit-callbacks
            (PE items that can fill exp-latency bubbles in the previous
            q-block's attention stream)."""
            qsl = slice(qb * QB, (qb + 1) * QB)
            psum_qk = pqk.tile([P, QB], F32, tag="qk", name=f"pqk{qb}")
            psum_kq = pqk.tile([P, QB], F32, tag="qk", name=f"pkq{qb}")
            items = []

            def mk_mm(ec, w_sb, psum, dst):
                def f():
                    nc.tensor.matmul(
                        psum[:], lhsT=w_sb[:, ec, :],
                        rhs=xp_sb[:, ec, qsl],
                        start=(ec == 0), stop=(ec == EC - 1))
                    if ec == EC - 1:
                        nc.vector.tensor_copy(dst[:, qsl], psum[:])
                return f
            # two packed projections: [Wq|Wk] -> qT@0:64,kT@64:128 and
            # [Wk|Wq] -> kT@0:64,qT@64:128 (gives both score-matmul
            # operand placements without any partition-shift DMAs)
            for ec in range(EC):
                items.append(mk_mm(ec, wqk_sb, psum_qk, qkT_sb))
            for ec in range(EC):
                items.append(mk_mm(ec, wkq_sb, psum_kq, kq2_sb))

            # v in natural layout directly: psum_v[s-tile, 64] +=
            # xp[e, s-tile].T @ Wv[e]  (lhsT = x block, small-N matmuls)
            def mk_v(t, e0, psum_v):
                def f():
                    for ec in range(e0, e0 + EC // 2):
                        nc.tensor.matmul(
                            psum_v[:],
                            lhsT=xp_sb[:, ec, t * P:(t + 1) * P],
                            rhs=wv_sb[:, ec, :],
                            start=(ec == 0), stop=(ec == EC - 1))
                    if e0 + EC // 2 == EC:
                        nc.vector.tensor_copy(v_sb[:, t, 0:H], psum_v[:])
                return f
            for t in range(qb * KPQ, (qb + 1) * KPQ):
                psum_v = pv.tile([P, H], F32, tag="v", name=f"psv{t}")
                items.append(mk_v(t, 0, psum_v))
                items.append(mk_v(t, EC // 2, psum_v))
            return items

        def emit_attention(qb, filler):
            """Attention for q-block qb; `filler` items are emitted
            between each score pair and its attn@v pair so the PE has
            independent work while ACT computes the exp."""
            qsl = slice(qb * QB, (qb + 1) * QB)
            nkc = (qb + 1) * KPQ
            npairs = nkc // 2
            psum_o = po.tile([H + 1, QB], F32)
            psum_o_pend[qb] = psum_o
            fill_idx = 0
            for pr in range(npairs):
                kc0, kc1 = 2 * pr, 2 * pr + 1
                # row-packed pair: kc0 on PE rows 0:64, kc1 on rows
                # 64:128 (kT lives at rows 64:128 of qkT_sb; qT
                # duplicated there); the two matmuls run concurrently
                o0 = max(0, kc0 * P - qb * QB)
                o1 = max(0, kc1 * P - qb * QB)
                psum_pr = ps.tile([P, 2, QB], F32, tag="sc")
                nc.tensor.matmul(
                    psum_pr[:, 0, o0:],
                    lhsT=kq2_sb[0:H, kc0 * P:(kc0 + 1) * P],
                    rhs=qkT_sb[0:H, qsl][:, o0:],
                    start=True, stop=True)
                nc.tensor.matmul(
                    psum_pr[:, 1, o1:],
                    lhsT=qkT_sb[H:P, kc1 * P:(kc1 + 1) * P],
                    rhs=kq2_sb[H:P, qsl][:, o1:],
                    start=True, stop=True)
                et = ep.tile([P, 2, QB], BF16)
                if o0 == o1:
                    # off-diagonal pair: one exp over both banks
                    nc.scalar.activation(
                        et[:, :, o0:], psum_pr[:, :, o0:],
                        mybir.ActivationFunctionType.Exp, scale=1.0 / H)
                else:
                    for i, o in ((0, o0), (1, o1)):
                        nc.scalar.activation(
                            et[:, i, o:], psum_pr[:, i, o:],
                            mybir.ActivationFunctionType.Exp, scale=1.0 / H)
                ets = []
                for i, (kc, o) in enumerate(((kc0, o0), (kc1, o1))):
                    if kc * P - qb * QB >= 0:
                        # diagonal chunk: keep where q >= k (j - p >= 0)
                        nc.gpsimd.affine_select(
                            out=et[:, i, o:], in_=et[:, i, o:],
                            compare_op=mybir.AluOpType.is_ge,
                            fill=0.0, base=0,
                            channel_multiplier=-1,
                            pattern=[[1, QB - o]])
                    ets.append((kc, o, et[:, i, :]))
                # fill the exp latency with next-block projection work
                take = ((pr + 1) * len(filler)) // npairs - fill_idx
                for _ in range(take):
                    filler[fill_idx]()
                    fill_idx += 1
                for kc, o, etv in ets:
                    nc.tensor.matmul(
                        psum_o[:, o:],
                        lhsT=v_sb[:, kc, :],
                        rhs=etv[:, o:],
                        start=(kc == 0), stop=(kc == nkc - 1))
            while fill_idx < len(filler):
                filler[fill_idx]()
                fill_idx += 1

        # q-block 0's projection is DMA-paced; emit it plainly
        for it in proj_items(0):
            it()
        for qb in range(NQB):
            filler = []
            if qb > 0:
                filler += normalize_items(qb - 1)
            if qb + 1 < NQB:
                filler += proj_items(qb + 1)
            emit_attention(qb, filler)
        for it in normalize_items(NQB - 1):
            it()


def build_bass(s=S, e_dim=E, n_cores=B):
    nc = bacc.Bacc(
        "TRN2", target_bir_lowering=False, debug=False, num_devices=n_cores)
    EC = e_dim // P
    xp_d = nc.dram_tensor("xp", [P, EC, s], BF16, kind="ExternalInput").ap()
    wqk_d = nc.dram_tensor(
        "wqk", [P, EC, 2 * H], BF16, kind="ExternalInput").ap()
    wkq_d = nc.dram_tensor(
        "wkq", [P, EC, 2 * H], BF16, kind="ExternalInput").ap()
    wv_d = nc.dram_tensor("wv", [P, EC, H], BF16, kind="ExternalInput").ap()
    out_d = nc.dram_tensor(
        "out", [P, s // P, H], F32, kind="ExternalOutput").ap()
    with tile.TileContext(nc) as tc:
        build_kernel_body(
            tc, xp_d, wqk_d, wkq_d, wv_d, out_d, s=s, e_dim=e_dim)
    nc.compile()
    return nc


_nc_cache = None


def _ensure_ntff_hook():
    """Dev-only: provide the antenv.axon_hooks shim so trace=True can
    capture NTFF profiles through libaxon_pjrt.so in this container."""
    import sys
    import types
    import ctypes
    import contextlib

    try:
        from antenv.axon_hooks import get_axon_ntff_profile_hook  # noqa
        return
    except ImportError:
        pass
    import antenv

    mod = types.ModuleType("antenv.axon_hooks")
    _h = [None]
    mod.set_axon_ntff_profile_hook = lambda h: _h.__setitem__(0, h)
    mod.get_axon_ntff_profile_hook = lambda: _h[0]
    sys.modules["antenv.axon_hooks"] = mod
    antenv.axon_hooks = mod

    so_path = "/opt/axon/libaxon_pjrt.so"
    lib = ctypes.CDLL(so_path)
    if not hasattr(lib, "axon_start_nrt_profile"):
        return
    lib.axon_start_nrt_profile.argtypes = [
        ctypes.POINTER(ctypes.c_int64), ctypes.c_size_t]
    lib.axon_start_nrt_profile.restype = ctypes.c_int64
    lib.axon_stop_nrt_profile.argtypes = [ctypes.c_char_p]
    lib.axon_stop_nrt_profile.restype = ctypes.c_int64

    @contextlib.contextmanager
    def _hook(output_dir, device_ids):
        import jax
        jax.devices()
        if device_ids:
            ids = (ctypes.c_int64 * len(device_ids))(*device_ids)
            rc = lib.axon_start_nrt_profile(ids, len(device_ids))
        else:
            rc = lib.axon_start_nrt_profile(None, 0)
        if rc != 0:
            raise RuntimeError(f"axon_start_nrt_profile rc={rc}")
        try:
            yield
        finally:
            n = lib.axon_stop_nrt_profile(str(output_dir).encode())
            print(f"profile: {n} file(s) written to {output_dir}")

    mod.set_axon_ntff_profile_hook(_hook)

    # no bucket access in this container; keep artifacts local
    import concourse.bass_utils as bu
    bu.upload_artifacts = lambda tmpdir: tmpdir


def _swizzle(a, ec, p):
    """[E, M] -> [P, EC, M] with [pp, c, m] = a[c*p + pp, m]."""
    return np.ascontiguousarray(a.reshape(ec, p, a.shape[-1]).transpose(1, 0, 2))


def kernel(x, Wq, Wk, Wv):
    global _nc_cache
    import ml_dtypes
    bf = ml_dtypes.bfloat16

    x = np.asarray(x, dtype=np.float32)
    Wq = np.asarray(Wq, dtype=np.float32)
    Wk = np.asarray(Wk, dtype=np.float32)
    Wv = np.asarray(Wv, dtype=np.float32)

    if _nc_cache is None:
        _nc_cache = build_bass()
    nc = _nc_cache

    EC = E // P
    wqk = _swizzle(np.concatenate([Wq, Wk], axis=1).astype(bf), EC, P)
    wkq = _swizzle(np.concatenate([Wk, Wq], axis=1).astype(bf), EC, P)
    wv = _swizzle(Wv.astype(bf), EC, P)
    in_maps = []
    for b in range(B):
        in_maps.append({
            "xp": _swizzle(x[b].T.astype(bf), EC, P),
            "wqk": wqk,
            "wkq": wkq,
            "wv": wv,
        })

    trace = bool(int(os.environ.get("ATTN_TRACE", "0")))
    if trace:
        _ensure_ntff_hook()
    res = run_bass_kernel_spmd(
        nc, in_maps, core_ids=list(range(B)), trace=trace)
    if trace and res.exec_time_ns is not None:
        print(f"HW exec time: {res.exec_time_ns} ns")
        kernel.last_exec_time_ns = res.exec_time_ns
        kernel.last_results = res
    # out [128, S//128, 64] per core -> [B, S, 64]
    out = np.stack(
        [np.ascontiguousarray(
            res.results[b]["out"].transpose(1, 0, 2).reshape(S, H))
         for b in range(B)],
        axis=0)
    return out
